# revision 1
# baseline (speedup 1.0000x reference)
"""CapsuleLayer dynamic-routing kernel for Trainium2 (8 NeuronCores).

Problem: B=256, I=2048, D=8 input capsules -> J=10, E=16 output capsules,
3 routing iterations.  Output = concat([v2, c2], axis=-1) -> [B, J, E+I].

Sharding: pure data parallelism over batch (32 batches/core), W replicated.

Per-core design (u_hat is NEVER materialized):
  s-steps:  s[b,j,e] = sum_{i,d} X[b,j,i,d] * W[j,i,e,d],  X = c (.) inputs
            PE matmuls, K=i (128-chunks), PSUM-accumulated over (chunk, d).
            Stationary wf[(i),(j,e)] slices, moving X[(i),(j,b)] -> diagonal
            j==j' entries of out[(j,e),(j',b)] are the result.
  t-steps:  t[b,j,i] = sum_e v[b,j,e] u_hat[b,j,i,e] computed as
            M1:   Y[(i,d), b]   = sum_e wm1[e,(i,d)]^T v[e,b]   (per j, PE)
            evac: Z[(i,d),(j,b)] = Y (.) inputs                  (DVE, PSUM->SBUF)
            M2:   blog[i, (j,b)] += ones_blkdiag^T Z             (PE, sums d)
  softmax over j without max-subtraction (logits are O(few)), ACT exp + DVE.

Layouts (i = ch*128 + p for the i-partition tensors; k = i*8 + d for (i,d)):
  wf    [128, 16, 8, 160]   wf[p,ch,d,j*16+e] = W[j, ch*128+p, e, d]
  wm1   [16, 10, 16384]     wm1[e,j,i*8+d]    = W[j, i, e, d]
  inp_i [128, 16, 32, 8]    inp_i[p,ch,b,d]   = inputs[b0+b, ch*128+p, d]
  inp_id[128, 128, 32]      inp_id[q,g,b]     = inputs[b0+b, g*16+q//8, q%8]
  ones  [128, 8, 128]       ones[q,gq,m]      = (m == 16*gq + q//8)
"""

import numpy as np

B, I, D, J, E = 256, 2048, 8, 10, 16
NCORES = 8
BL = B // NCORES          # 32 batches per core
NCH = I // 128            # 16 i-chunks of 128
NG = (I * D) // 128       # 128 (i,d)-groups of 128
JE = J * E                # 160
JB = J * BL               # 320  (j,b) column count
OUTW = E + I              # 2064
EPS = 1e-7

_PROGRAM = None


def _host_prep(inputs, W):
    """Build all DRAM-side arrays. Returns (shared dict, per-core list)."""
    W = np.ascontiguousarray(W, dtype=np.float32)
    inputs = np.ascontiguousarray(inputs, dtype=np.float32)

    # wf[p, ch, d, j*16+e] = W[j, ch*128+p, e, d]
    wf = W.transpose(1, 3, 0, 2).reshape(NCH, 128, D, JE)  # [ch,p,d,(j,e)] wait
    # W.transpose(1,3,0,2): [I, D, J, E] -> index [i, d, j, e]
    # reshape(NCH,128,D,J*E) splits i -> (ch, p): [ch, p, d, (j,e)]
    wf = np.ascontiguousarray(wf.transpose(1, 0, 2, 3))   # [p, ch, d, (j,e)]

    # wm1[e, j, i*8+d] = W[j, i, e, d]
    wm1 = np.ascontiguousarray(W.transpose(2, 0, 1, 3).reshape(E, J, I * D))

    # ones[q, gq, m] = 1 iff m == 16*gq + q//8
    ones = np.zeros((128, 8, 128), dtype=np.float32)
    q = np.arange(128)
    for gq in range(8):
        ones[q, gq, 16 * gq + q // 8] = 1.0

    shared = {"wf": wf, "wm1": wm1, "ones": ones}

    per_core = []
    for m in range(NCORES):
        sl = inputs[m * BL:(m + 1) * BL]                  # [32, 2048, 8]
        # inp_i[p, ch, b, d] = sl[b, ch*128+p, d]
        inp_i = np.ascontiguousarray(
            sl.reshape(BL, NCH, 128, D).transpose(2, 1, 0, 3))
        # inp_id[q, g, b] = sl[b, g*16 + q//8, q%8]
        inp_id = np.ascontiguousarray(
            sl.reshape(BL, NG, 16, 8).transpose(2, 3, 1, 0).reshape(128, NG, BL))
        per_core.append({"inp_i": inp_i, "inp_id": inp_id})
    return shared, per_core


def _build_program(debug=False):
    from contextlib import ExitStack
    import concourse.mybir as mybir
    from concourse import bacc
    from concourse.tile import TileContext

    f32 = mybir.dt.float32
    nc = bacc.Bacc()

    wf_d = nc.dram_tensor("wf", [128, NCH, D, JE], f32, kind="ExternalInput")
    wm1_d = nc.dram_tensor("wm1", [E, J, I * D], f32, kind="ExternalInput")
    ones_d = nc.dram_tensor("ones", [128, 8, 128], f32, kind="ExternalInput")
    inpi_d = nc.dram_tensor("inp_i", [128, NCH, BL, D], f32, kind="ExternalInput")
    inpid_d = nc.dram_tensor("inp_id", [128, NG, BL], f32, kind="ExternalInput")
    out_d = nc.dram_tensor("out", [BL, J, OUTW], f32, kind="ExternalOutput")

    _kernel_body.debug_tensors = {}
    if debug:
        _kernel_body.debug_tensors = {
            "s0": nc.dram_tensor("dbg_s0", [BL, J, 32], f32, kind="ExternalOutput"),
            "v0": nc.dram_tensor("dbg_v0", [E, J, BL], f32, kind="ExternalOutput"),
            "blog0": nc.dram_tensor("dbg_blog0", [128, NCH, J, BL], f32,
                                    kind="ExternalOutput"),
            "c1": nc.dram_tensor("dbg_c1", [128, NCH, J, BL], f32,
                                 kind="ExternalOutput"),
            "x0": nc.dram_tensor("dbg_x0", [128, D, J, BL], f32,
                                 kind="ExternalOutput"),
            "psa0": nc.dram_tensor("dbg_psa0", [128, 8, BL], f32,
                                   kind="ExternalOutput"),
            "psb0": nc.dram_tensor("dbg_psb0", [32, 2, BL], f32,
                                   kind="ExternalOutput"),
        }

    with ExitStack() as ctx:
        tc = ctx.enter_context(TileContext(nc))
        _kernel_body(ctx, tc, wf_d, wm1_d, ones_d, inpi_d, inpid_d, out_d)
    nc.compile()
    return nc


def _kernel_body(ctx, tc, wf_d, wm1_d, ones_d, inpi_d, inpid_d, out_d):
    import concourse.bass as bass
    import concourse.mybir as mybir

    f32 = mybir.dt.float32
    nc = tc.nc
    AF = mybir.ActivationFunctionType
    ALU = mybir.AluOpType
    AX = mybir.AxisListType

    # ---------------- pools ----------------
    const = ctx.enter_context(tc.tile_pool(name="const", bufs=1))
    state = ctx.enter_context(tc.tile_pool(name="state", bufs=1))
    xpool = ctx.enter_context(tc.tile_pool(name="xpool", bufs=2))
    wstg = ctx.enter_context(tc.tile_pool(name="wstg", bufs=3))
    zpool = ctx.enter_context(tc.tile_pool(name="zpool", bufs=2))
    small = ctx.enter_context(tc.tile_pool(name="small", bufs=2))
    ps_s = ctx.enter_context(tc.tile_pool(name="ps_s", bufs=1, space="PSUM"))
    ps_y = ctx.enter_context(tc.tile_pool(name="ps_y", bufs=2, space="PSUM"))
    ps_b = ctx.enter_context(tc.tile_pool(name="ps_b", bufs=2, space="PSUM"))

    # ---------------- resident loads ----------------
    wf = const.tile([128, NCH, D, JE], f32)
    for ch in range(NCH):
        nc.sync.dma_start(out=wf[:, ch], in_=wf_d[:, ch])
    inp_i = const.tile([128, NCH, BL, D], f32)
    nc.sync.dma_start(out=inp_i[:], in_=inpi_d[:])
    inp_id = const.tile([128, NG, BL], f32)
    nc.sync.dma_start(out=inp_id[:], in_=inpid_d[:])
    ones = const.tile([128, 8, 128], f32)
    nc.sync.dma_start(out=ones[:], in_=ones_d[:])
    epsb = const.tile([BL, 1], f32)
    nc.vector.memset(epsb[:], EPS)

    # persistent state
    blog = state.tile([128, NCH, J, BL], f32)   # routing logits, [i, (j,b)]
    cbuf = state.tile([128, NCH, J, BL], f32)   # coupling coeffs c
    # s/v in b-partition layout during squash; vbufx rows 0:16 hold v [e,j,b]
    sbT2 = state.tile([BL, J, 32], f32)         # transposed s (+garbage cols)
    vT = state.tile([BL, J, 32], f32)           # v in b-part (+garbage cols)
    vbufx = state.tile([32, J, BL], f32)        # v [e(0:16), j, b] for M1
    vbuf2 = state.tile([E, J, BL], f32)         # v compacted to base-0
    s2T = state.tile([BL, 5, 2, E], f32)
    nrmT = state.tile([BL, 5, 2], f32)
    sclT = state.tile([BL, 5, 2], f32)
    tmpT = state.tile([BL, 5, 2], f32)

    def valid_view(tile_ap):
        """[BL, J, 32] -> strided [BL, 5, 2, 16] view of the valid e-cols.

        Valid cols of j=2q+jj sit at flat offset 64q + 48jj (steps 64/48/1),
        expressed as a step-3 slice over 16-wide chunks.
        """
        return tile_ap.rearrange("b j e -> b (j e)") \
            .rearrange("b (q c s) -> b q c s", q=5, c=4, s=16)[:, :, 0::3, :]

    def squash(iter0):
        """psA/psB diag -> (transpose) -> squash in b-part -> vbufx [e,j,b].

        True s = 0.1*s_raw on iter0: n_true = 0.01*n_raw,
        v = squash_scale(n_true) * 0.1 * s_raw.
        """
        sAP = valid_view(sbT2[:])
        nc.scalar.square(s2T[:], sAP)
        nc.vector.tensor_reduce(nrmT[:], s2T[:], AX.X, ALU.add)
        k = 0.01 if iter0 else 1.0
        # tmpT = 1/(1 + k*n)
        nc.scalar.activation(tmpT[:], nrmT[:], AF.Copy, scale=k)
        nc.vector.tensor_scalar_add(tmpT[:], tmpT[:], 1.0)
        nc.vector.reciprocal(tmpT[:], tmpT[:])
        # sclT = 1/sqrt(k*n + eps)
        nc.scalar.activation(sclT[:], nrmT[:], AF.Sqrt, scale=k, bias=epsb[:])
        nc.vector.reciprocal(sclT[:], sclT[:])
        # sclT = k*n * tmpT * sclT * (0.1 iter0)
        nc.vector.tensor_mul(sclT[:], sclT[:], tmpT[:])
        kk = k * (0.1 if iter0 else 1.0)
        nc.scalar.activation(sclT[:], sclT[:], AF.Copy, scale=kk)
        nc.vector.tensor_mul(sclT[:], sclT[:], nrmT[:])
        # vT = s * scale (broadcast over e), on the valid cols view
        nc.vector.tensor_tensor(
            valid_view(vT[:]),
            sAP,
            sclT[:].unsqueeze(3).broadcast_to([BL, 5, 2, 16]),
            ALU.mult)
        # transpose back: valid v of j lands at vbufx rows 16*(j%2)+0:16;
        # compact to base-0 via SBUF->SBUF DMA (engines are lane-locked,
        # DMA is address-based so it can shift partitions)
        for j in range(J):
            nc.vector.transpose(vbufx[:, j], vT[:, j])
        for j in range(J):
            r = 16 * (j % 2)
            nc.sync.dma_start(out=vbuf2[:, j], in_=vbufx[r:r + 16, j])

    def v_ap(j):
        """M1/moving view of v for capsule j: [16, BL]."""
        return vbuf2[:, j]

    def s_step(it):
        """cbuf (or uniform 0.1 if it==0) -> s matmuls -> sbuf_s [E,J,BL].

        Per (ch, d, j): psS[e, j, b] += wf[i,(j,e)]^T X[i, (j,b)],
        PSUM-accumulated over the 128 (ch,d) pairs.
        """
        psA = ps_s.tile([128, 8, BL], f32, name=f"psA{it}", tag="psA")
        psB = ps_s.tile([32, 2, BL], f32, name=f"psB{it}", tag="psB")
        nmm = NCH * D
        k = 0
        for ch in range(NCH):
            X = xpool.tile([128, D, J, BL], f32, name=f"X{it}_{ch}", tag="X")
            if it == 0:
                src = inp_i[:, ch].rearrange("p b d -> p d b") \
                    .unsqueeze(2).broadcast_to([128, D, J, BL])
                nc.gpsimd.tensor_scalar_mul(X[:], src, 1.0)
            else:
                cin = cbuf[:, ch].unsqueeze(1).broadcast_to([128, D, J, BL])
                iin = inp_i[:, ch].rearrange("p b d -> p d b") \
                    .unsqueeze(2).broadcast_to([128, D, J, BL])
                nc.gpsimd.tensor_tensor(X[:], cin, iin, ALU.mult)
            dbg = _kernel_body.debug_tensors
            if it == 0 and ch == 0 and "x0" in dbg:
                nc.sync.dma_start(out=dbg["x0"][:], in_=X[:])
            for d in range(D):
                st = (k == 0)
                sp = (k == nmm - 1)
                nc.tensor.matmul(
                    psA[:].rearrange("p j b -> p (j b)"),
                    wf[:, ch, d, 0:128],
                    X[:, d, 0:8].rearrange("p j b -> p (j b)"),
                    start=st, stop=sp)
                nc.tensor.matmul(
                    psB[:].rearrange("p j b -> p (j b)"),
                    wf[:, ch, d, 128:160],
                    X[:, d, 8:10].rearrange("p j b -> p (j b)"),
                    start=st, stop=sp)
                k += 1
        if it == 0 and "psa0" in _kernel_body.debug_tensors:
            dbg = _kernel_body.debug_tensors
            pacopy = small.tile([128, 8, BL], f32, name="pacopy", tag="pac")
            nc.vector.tensor_copy(pacopy[:], psA[:])
            nc.sync.dma_start(out=dbg["psa0"][:], in_=pacopy[:])
            pbcopy = small.tile([32, 2, BL], f32, name="pbcopy", tag="pbc")
            nc.vector.tensor_copy(pbcopy[:], psB[:])
            nc.sync.dma_start(out=dbg["psb0"][:], in_=pbcopy[:])
        # diagonal extract via 32x32 DVE transposes (PSUM compute reads must
        # be 32-partition aligned; each transpose grabs a j-pair's rows and
        # lands s[b, e] in b-partition layout, valid cols at 16*(j%2))
        for q in range(4):
            for jj in range(2):
                j = 2 * q + jj
                nc.vector.transpose(sbT2[:, j], psA[32 * q:32 * (q + 1), j])
        nc.vector.transpose(sbT2[:, 8], psB[:, 0])
        nc.vector.transpose(sbT2[:, 9], psB[:, 1])

    def t_step(it):
        """vbuf -> blog (it==0: overwrite; else accumulate)."""
        for sup in range(NCH):
            bp = ps_b.tile([128, J, BL], f32, name=f"bp{it}_{sup}", tag="bp")
            for gq in range(8):
                g = sup * 8 + gq
                stg = wstg.tile([E, J, 128], f32, name=f"wst{it}_{g}", tag="wst")
                nc.sync.dma_start(out=stg[:], in_=wm1_d[:, :, 128 * g:128 * (g + 1)])
                yp = ps_y.tile([128, J, BL], f32, name=f"yp{it}_{g}", tag="yp")
                for j in range(J):
                    nc.tensor.matmul(yp[:, j], stg[:, j], v_ap(j))
                Z = zpool.tile([128, J, BL], f32, name=f"Z{it}_{g}", tag="Z")
                nc.vector.tensor_tensor(
                    Z[:], yp[:],
                    inp_id[:, g].unsqueeze(1).broadcast_to([128, J, BL]),
                    ALU.mult)
                nc.tensor.matmul(bp[:].rearrange("p j b -> p (j b)"),
                                 ones[:, gq],
                                 Z[:].rearrange("p j b -> p (j b)"),
                                 start=(gq == 0), stop=(gq == 7))
            if it == 0:
                nc.scalar.copy(blog[:, sup], bp[:])
            else:
                nc.vector.tensor_add(blog[:, sup], blog[:, sup], bp[:])

    def softmax():
        """cbuf = softmax_j(blog) (no max-subtraction; logits are small)."""
        for ch in range(NCH):
            nc.scalar.activation(cbuf[:, ch], blog[:, ch], AF.Exp)
            ssum = small.tile([128, BL], f32, name=f"ss{ch}", tag="ssum")
            nc.vector.tensor_reduce(
                ssum[:], cbuf[:, ch].rearrange("p j b -> p b j"),
                AX.X, ALU.add)
            nc.vector.reciprocal(ssum[:], ssum[:])
            nc.vector.tensor_mul(
                cbuf[:, ch], cbuf[:, ch],
                ssum[:].unsqueeze(1).broadcast_to([128, J, BL]))

    # ---------------- the routing schedule ----------------
    dbg = _kernel_body.debug_tensors
    s_step(0)
    if "s0" in dbg:
        nc.sync.dma_start(out=dbg["s0"][:], in_=sbT2[:])
    squash(True)          # v0
    if "v0" in dbg:
        nc.sync.dma_start(out=dbg["v0"][:], in_=vbuf2[:])
    t_step(0)             # blog = t0
    if "blog0" in dbg:
        nc.sync.dma_start(out=dbg["blog0"][:], in_=blog[:])
    softmax()             # c1
    if "c1" in dbg:
        nc.sync.dma_start(out=dbg["c1"][:], in_=cbuf[:])
    s_step(1)
    squash(False)         # v1
    t_step(1)             # blog += t1
    softmax()             # c2
    s_step(2)
    squash(False)         # v2

    # ---------------- output ----------------
    # out[b, j, 0:16] = v2[e, j, b]
    for j in range(J):
        nc.sync.dma_start(out=out_d[:, j, 0:E].rearrange("b e -> e b"),
                          in_=v_ap(j))
    # out[b, j, 16:2064] = c2[b, j, i], i = ch*128 + p
    for j in range(J):
        for b in range(BL):
            nc.sync.dma_start(
                out=out_d[b, j, E:OUTW].rearrange("(c p) -> p c", p=128),
                in_=cbuf[:, :, j, b])


def kernel(inputs, W):
    global _PROGRAM
    from concourse.bass_utils import run_bass_kernel_spmd

    shared, per_core = _host_prep(np.asarray(inputs), np.asarray(W))
    if _PROGRAM is None:
        _PROGRAM = _build_program()
    in_maps = [{**shared, **pc} for pc in per_core]
    res = run_bass_kernel_spmd(_PROGRAM, in_maps, core_ids=list(range(NCORES)))
    out = np.concatenate([r["out"] for r in res.results], axis=0)
    return out.astype(np.float32)


if __name__ == "__main__":
    rng = np.random.default_rng(0)
    x = rng.standard_normal((B, I, D), dtype=np.float32)
    w = rng.standard_normal((J, I, E, D), dtype=np.float32)
    y = kernel(x, w)
    print(y.shape, y.dtype)



# revision 12
# speedup vs baseline: 9.8544x; 9.8544x over previous
"""CapsuleLayer dynamic-routing kernel for Trainium2 (8 NeuronCores), v2.

Problem: B=256, I=2048, D=8 input capsules -> J=10, E=16 output capsules,
3 routing iterations.  Output = concat([v2, c2], axis=-1) -> [B, J, E+I].

Sharding: pure data parallelism over batch (32 batches/core), W replicated.

Per-core design (u_hat never materialized; all matmuls bf16, fp32 PSUM):

  s-step:  s[b,j,e] = sum_{i,d} X[b,j,i,d] W[j,i,e,d],  X = c (.) inputs
           PE: per (ch,d) chunk k=128 i's:  psA[(j,e){j<8}, (j',b)] and
           psB[(j8:10,e), (j',b)] accumulate over 128 chunks; diagonal
           j==j' holds s.  it0: X = inputs broadcast over j (c uniform,
           0.1 scale folded into squash via k=0.01).
  squash:  diag extract via DVE 32x32 transposes into b-partition layout
           [32, J, 32], fp32 squash there, transpose back into the
           block-diagonal moving operand vmovA[(j,e){j<8}, (j',b)] /
           vmovC[(j8:10,e), (j',b)] (bf16, off-diag zeros persist).
  t-step:  t[b,j,i] = sum_e v[b,j,e] u_hat[b,j,i,e], computed per
           i-group g (16 i's, q=(i%16)*8+d):
             Y:  psY[q, (j,b)] = wtA[:,g].T @ vmovA  (+ wtC part)
             evac: Ycp = ACT copy psY -> bf16 SBUF
             Z = Ycp (.) inp_qb[:,g] (DVE, broadcast over j)
             M2: psT[i16 placed, (j,b)] += ones[:,g%8].T @ Z  (sums d)
           psT per sup (8 g's) -> blog[i, sup, (j,b)]  (copy/add).
  softmax: eb = exp(blog) (ACT, bf16), esum/recip (DVE), X for the next
           s-step = eb (.) (r*inputs) -- c itself only materialized for
           the final output.

Layouts (i = ch*128 + p for s-step tensors; q = (i%16)*8 + d, g = i//16):
  wfb    [128, 16, 8, 160]  bf16  wfb[p,ch,d,j*16+e] = W[j, ch*128+p, e, d]
  wtA    [128, 128, 128]    bf16  wtA[j*16+e, g, q]  = W[j, g*16+q//8, e, q%8]  (j<8)
  wtC    [32, 128, 128]     bf16  same for j in {8,9}
  ones   [128, 8, 128]      bf16  ones[q,gq,m] = (m == 16*gq + q//8)
  inp_dib[128, 16, 8, 32]   bf16  inp_dib[p,ch,d,b] = inputs[b0+b, ch*128+p, d]
  inp_qb [128, 128, 32]     bf16  inp_qb[q,g,b]     = inputs[b0+b, g*16+q//8, q%8]
  out_c  [128, 16, 10, 32]  f32   out_c[p,ch,j,b]   = c2[b, j, ch*128+p]
  out_v  [32, 10, 16]       f32   out_v[b,j,e]      = v2[b, j, e]
"""

import numpy as np
import ml_dtypes

B, I, D, J, E = 256, 2048, 8, 10, 16
NCORES = 8
BL = B // NCORES          # 32 batches per core
NCH = I // 128            # 16 i-chunks of 128 (s-step)
NG = I // 16              # 128 i-groups of 16 (t-step)
JE = J * E                # 160
OUTW = E + I              # 2064
EPS = 1e-7

BF16 = ml_dtypes.bfloat16
_PROGRAM = None


def _host_prep(inputs, W):
    """Build all DRAM-side arrays. Returns (shared dict, per-core list)."""
    W = np.ascontiguousarray(W, dtype=np.float32)
    inputs = np.ascontiguousarray(inputs, dtype=np.float32)

    # wfb[p, ch, d, j*16+e] = W[j, ch*128+p, e, d]
    wfb = W.transpose(1, 3, 0, 2).reshape(NCH, 128, D, JE)   # [ch, p, d, (j,e)]
    wfb = np.ascontiguousarray(wfb.transpose(1, 0, 2, 3)).astype(BF16)

    # wtA[j*16+e, g, q] = W[j, g*16+q//8, e, q%8]
    wt = W.transpose(0, 2, 1, 3)                             # [J, E, I, D]
    wt = wt.reshape(J * E, NG, 16 * D)                       # [(j,e), g, q]
    wtA = np.ascontiguousarray(wt[0:128]).astype(BF16)
    # wtC packed 4 g-groups deep: wtC4[32*(g%4)+(j,e), g//4, q]
    wtC = wt[128:160].reshape(32, NG // 4, 4, 16 * D)        # [(j,e), g4, gm, q]
    wtC4 = np.ascontiguousarray(
        wtC.transpose(2, 0, 1, 3).reshape(128, NG // 4, 16 * D)).astype(BF16)

    # ones[q, gq, m] = 1 iff m == 16*gq + q//8
    ones = np.zeros((128, 8, 128), dtype=np.float32)
    q = np.arange(128)
    for gq in range(8):
        ones[q, gq, 16 * gq + q // 8] = 1.0
    ones = ones.astype(BF16)

    shared = {"wfb": wfb, "wtA": wtA, "wtC": wtC4, "ones": ones}

    per_core = []
    for m in range(NCORES):
        sl = inputs[m * BL:(m + 1) * BL]                     # [32, 2048, 8]
        # inp_dib[p, ch, d, b] = sl[b, ch*128+p, d]
        inp_dib = np.ascontiguousarray(
            sl.reshape(BL, NCH, 128, D).transpose(2, 1, 3, 0)).astype(BF16)
        # inp_qb[q, g, b] = sl[b, g*16 + q//8, q%8]
        inp_qb = np.ascontiguousarray(
            sl.reshape(BL, NG, 16 * D).transpose(2, 1, 0)).astype(BF16)
        per_core.append({"inp_dib": inp_dib, "inp_qb": inp_qb})
    return shared, per_core


def _build_program():
    from contextlib import ExitStack
    import concourse.mybir as mybir
    from concourse import bacc
    from concourse.tile import TileContext

    f32 = mybir.dt.float32
    bf16 = mybir.dt.bfloat16
    nc = bacc.Bacc()

    wfb_d = nc.dram_tensor("wfb", [128, NCH, D, JE], bf16, kind="ExternalInput")
    wtA_d = nc.dram_tensor("wtA", [128, NG, 128], bf16, kind="ExternalInput")
    wtC_d = nc.dram_tensor("wtC", [128, NG // 4, 128], bf16,
                             kind="ExternalInput")
    ones_d = nc.dram_tensor("ones", [128, 8, 128], bf16, kind="ExternalInput")
    inpdib_d = nc.dram_tensor("inp_dib", [128, NCH, D, BL], bf16,
                              kind="ExternalInput")
    inpqb_d = nc.dram_tensor("inp_qb", [128, NG, BL], bf16, kind="ExternalInput")
    outc_d = nc.dram_tensor("out_c", [128, NCH, J, BL], f32,
                            kind="ExternalOutput")
    outv_d = nc.dram_tensor("out_v", [BL, J, E], f32, kind="ExternalOutput")

    with ExitStack() as ctx:
        tc = ctx.enter_context(TileContext(nc))
        _kernel_body(ctx, tc, wfb_d, wtA_d, wtC_d, ones_d, inpdib_d, inpqb_d,
                     outc_d, outv_d)
    nc.compile()
    return nc


def _kernel_body(ctx, tc, wfb_d, wtA_d, wtC_d, ones_d, inpdib_d, inpqb_d,
                 outc_d, outv_d):
    import concourse.mybir as mybir

    f32 = mybir.dt.float32
    bf16 = mybir.dt.bfloat16
    nc = tc.nc
    AF = mybir.ActivationFunctionType
    ALU = mybir.AluOpType
    AX = mybir.AxisListType

    # ---------------- pools ----------------
    const = ctx.enter_context(tc.tile_pool(name="const", bufs=1))
    state = ctx.enter_context(tc.tile_pool(name="state", bufs=1))
    xpool = ctx.enter_context(tc.tile_pool(name="xpool", bufs=2))
    zpool = ctx.enter_context(tc.tile_pool(name="zpool", bufs=4))
    ypool = ctx.enter_context(tc.tile_pool(name="ypool", bufs=4))
    small = ctx.enter_context(tc.tile_pool(name="small", bufs=2))
    ps_s = ctx.enter_context(tc.tile_pool(name="ps_s", bufs=1, space="PSUM"))
    ps_y = ctx.enter_context(tc.tile_pool(name="ps_y", bufs=3, space="PSUM"))
    ps_t = ctx.enter_context(tc.tile_pool(name="ps_t", bufs=2, space="PSUM"))

    # ---------------- resident loads ----------------
    wfb = const.tile([128, NCH, D, JE], bf16)
    nc.sync.dma_start(out=wfb[:], in_=wfb_d[:])
    inp_dib = const.tile([128, NCH, D, BL], bf16)
    nc.sync.dma_start(out=inp_dib[:], in_=inpdib_d[:])
    inp_qb = const.tile([128, NG, BL], bf16)
    nc.sync.dma_start(out=inp_qb[:], in_=inpqb_d[:])
    wtA = const.tile([128, NG, 128], bf16)
    nc.sync.dma_start(out=wtA[:], in_=wtA_d[:])
    wtC = const.tile([128, NG // 4, 128], bf16)
    nc.sync.dma_start(out=wtC[:], in_=wtC_d[:])
    ones = const.tile([128, 8, 128], bf16)
    nc.sync.dma_start(out=ones[:], in_=ones_d[:])
    epsb = const.tile([BL, 1], f32)
    nc.vector.memset(epsb[:], EPS)

    # persistent state
    blog = state.tile([128, NCH, J, BL], f32)     # routing logits [i,(sup,j,b)]
    eb = state.tile([128, NCH, J, BL], bf16)      # exp(blog)
    rs = state.tile([128, NCH, BL], f32)          # 1/sum_j eb
    ri = state.tile([128, NCH, D, BL], bf16)      # rs * inputs
    vmovA = state.tile([128, 8, BL], bf16)        # block-diag v, j 0..7
    vmovC = state.tile([128, 2, BL], bf16)        # block-diag v, j 8..9, x4
    sbT = state.tile([BL, J, 32], f32)            # s in b-partition (+garbage)
    vT = state.tile([BL, J, 32], f32)             # v in b-partition (+zeros)
    vstg = state.tile([128, 2, 32], f32)          # f32 staging for vmovA
    vstgC = state.tile([128, 2, 32], f32)         # f32 staging for vmovC
    s2T = state.tile([BL, 5, 2, E], f32)
    nrmT = state.tile([BL, 5, 2], f32)
    sclT = state.tile([BL, 5, 2], f32)
    tmpT = state.tile([BL, 5, 2], f32)

    nc.vector.memset(vmovA[:], 0.0)
    nc.vector.memset(vmovC[:], 0.0)
    nc.vector.memset(vT[:], 0.0)

    def valid_view(tile_ap):
        """[BL, J, 32] -> strided [BL, 5, 2, 16] view of the valid e-cols.

        Valid cols of j=2q+jj sit at flat offset 64q + 48jj (steps 64/48/1),
        expressed as a step-3 slice over 16-wide chunks.
        """
        return tile_ap.rearrange("b j e -> b (j e)") \
            .rearrange("b (q c s) -> b q c s", q=5, c=4, s=16)[:, :, 0::3, :]

    def s_step(it):
        """X (or inputs if it==0) -> psA/psB -> sbT diag extract."""
        psA = ps_s.tile([128, 8, BL], f32, name=f"psA{it}", tag="psA")
        psB = ps_s.tile([32, 2, BL], f32, name=f"psB{it}", tag="psB")
        nmm = NCH * D
        k = 0
        for ch in range(NCH):
            if it == 0:
                X = None
            else:
                X = xpool.tile([128, D, J, BL], bf16, name=f"X{it}_{ch}",
                               tag="X")
                nc.vector.tensor_tensor(
                    X[:],
                    eb[:, ch].unsqueeze(1).broadcast_to([128, D, J, BL]),
                    ri[:, ch].unsqueeze(2).broadcast_to([128, D, J, BL]),
                    ALU.mult)
            for d in range(D):
                st = (k == 0)
                sp = (k == nmm - 1)
                if it == 0:
                    rhsA = inp_dib[:, ch, d].unsqueeze(1) \
                        .broadcast_to([128, 8, BL])
                    rhsB = inp_dib[:, ch, d].unsqueeze(1) \
                        .broadcast_to([128, 2, BL])
                else:
                    rhsA = X[:, d, 0:8]
                    rhsB = X[:, d, 8:10]
                nc.tensor.matmul(psA[:], wfb[:, ch, d, 0:128], rhsA,
                                 start=st, stop=sp)
                nc.tensor.matmul(psB[:], wfb[:, ch, d, 128:160], rhsB,
                                 start=st, stop=sp)
                k += 1
        # diagonal extract via 32x32 DVE transposes into b-partition layout
        for q in range(4):
            for jj in range(2):
                j = 2 * q + jj
                nc.vector.transpose(sbT[:, j], psA[32 * q:32 * (q + 1), j])
        nc.vector.transpose(sbT[:, 8], psB[:, 0])
        nc.vector.transpose(sbT[:, 9], psB[:, 1])

    def squash(iter0, last):
        """sbT -> squash in b-partition -> vT; rebuild vmovA/vmovC (not last).

        True s = 0.1*s_raw on iter0: n_true = 0.01*n_raw,
        v = squash_scale(n_true) * 0.1 * s_raw.
        """
        sAP = valid_view(sbT[:])
        nc.scalar.square(s2T[:], sAP)
        nc.vector.tensor_reduce(nrmT[:], s2T[:], AX.X, ALU.add)
        kk = 0.01 if iter0 else 1.0
        # tmpT = 1/(1 + k*n)
        nc.scalar.activation(tmpT[:], nrmT[:], AF.Copy, scale=kk)
        nc.vector.tensor_scalar_add(tmpT[:], tmpT[:], 1.0)
        nc.vector.reciprocal(tmpT[:], tmpT[:])
        # sclT = 1/sqrt(k*n + eps)
        nc.scalar.activation(sclT[:], nrmT[:], AF.Sqrt, scale=kk, bias=epsb[:])
        nc.vector.reciprocal(sclT[:], sclT[:])
        # sclT = k'*n * tmpT * sclT,  k' = k * (0.1 iter0)
        nc.vector.tensor_mul(sclT[:], sclT[:], tmpT[:])
        k2 = kk * (0.1 if iter0 else 1.0)
        nc.scalar.activation(sclT[:], sclT[:], AF.Copy, scale=k2)
        nc.vector.tensor_mul(sclT[:], sclT[:], nrmT[:])
        # vT = s * scale (broadcast over e), on the valid cols view
        nc.vector.tensor_tensor(
            valid_view(vT[:]),
            sAP,
            sclT[:].unsqueeze(3).broadcast_to([BL, 5, 2, 16]),
            ALU.mult)
        if last:
            return
        # transpose back per j into f32 staging (transpose can't cast), then
        # cast-copy the [32, 2, 32] block pair into the bf16 operands.
        # vT garbage cols are zero, so each 32x32 transpose emits the valid
        # v rows plus zeros -- exactly one (j,e)-row x (j',b)-col block.
        for q in range(4):
            lo, hi = 32 * q, 32 * (q + 1)
            nc.vector.transpose(vstg[lo:hi, 0], vT[:, 2 * q])
            nc.vector.transpose(vstg[lo:hi, 1], vT[:, 2 * q + 1])
            nc.vector.tensor_copy(vmovA[lo:hi, 2 * q:2 * q + 2], vstg[lo:hi, :])
        for m in range(0, 128, 32):
            nc.vector.transpose(vstgC[m:m + 32, 0], vT[:, 8])
            nc.vector.transpose(vstgC[m:m + 32, 1], vT[:, 9])
            nc.vector.tensor_copy(vmovC[m:m + 32, :], vstgC[m:m + 32, :])

    def t_step(it):
        """vmov -> psY per i-group -> Z -> psT (ones matmul) -> blog."""
        for sup in range(NCH):
            psT = ps_t.tile([128, J, BL], f32, name=f"psT{it}_{sup}", tag="psT")
            for gq in range(8):
                g = sup * 8 + gq
                psY = ps_y.tile([128, J, BL], f32, name=f"psY{it}_{g}",
                                tag="psY")
                nc.tensor.matmul(psY[:, 0:8], wtA[:, g], vmovA[:],
                                 start=True, stop=True)
                m = 32 * (g % 4)
                nc.tensor.matmul(psY[:, 8:10], wtC[m:m + 32, g // 4],
                                 vmovC[m:m + 32], start=True, stop=True,
                                 tile_position=(m, 0))
                ycp = ypool.tile([128, J, BL], bf16, name=f"y{it}_{g}", tag="y")
                nc.scalar.copy(ycp[:], psY[:])
                Z = zpool.tile([128, J, BL], bf16, name=f"Z{it}_{g}", tag="Z")
                nc.vector.tensor_tensor(
                    Z[:], ycp[:],
                    inp_qb[:, g].unsqueeze(1).broadcast_to([128, J, BL]),
                    ALU.mult)
                nc.tensor.matmul(psT[:], ones[:, gq], Z[:],
                                 start=(gq == 0), stop=(gq == 7))
            if it == 0:
                nc.vector.tensor_copy(blog[:, sup], psT[:])
            else:
                nc.vector.tensor_add(blog[:, sup], blog[:, sup], psT[:])

    def softmax(last):
        """blog -> eb, rs, ri (and out_c on the last call)."""
        nc.scalar.activation(eb[:], blog[:], AF.Exp)
        esum = small.tile([128, NCH, BL], f32, name="esum", tag="esum")
        nc.vector.tensor_reduce(
            esum[:], eb[:].rearrange("p c j b -> p c b j"), AX.X, ALU.add)
        nc.vector.reciprocal(rs[:], esum[:])
        nc.vector.tensor_tensor(
            ri[:], inp_dib[:],
            rs[:].unsqueeze(2).broadcast_to([128, NCH, D, BL]),
            ALU.mult)
        if last:
            for c0 in range(0, NCH, 4):
                cout = small.tile([128, 4, J, BL], f32, name=f"co{c0}",
                                  tag="cout")
                nc.vector.tensor_tensor(
                    cout[:], eb[:, c0:c0 + 4],
                    rs[:, c0:c0 + 4].unsqueeze(2)
                    .broadcast_to([128, 4, J, BL]),
                    ALU.mult)
                nc.sync.dma_start(out=outc_d[:, c0:c0 + 4], in_=cout[:])

    # ---------------- the routing schedule ----------------
    s_step(0)
    squash(True, False)       # v0
    t_step(0)                 # blog = t0
    softmax(False)            # c1 (as eb/rs/ri)
    s_step(1)
    squash(False, False)      # v1
    t_step(1)                 # blog += t1
    softmax(True)             # c2 -> out_c
    s_step(2)
    squash(False, True)       # v2 -> vT

    # out_v[b, j, e] = vT valid cols (compact first; strided DMA unbalanceable)
    vout = state.tile([BL, J, E], f32)
    nc.vector.tensor_copy(vout[:].rearrange("b (q c) e -> b q c e", q=5),
                          valid_view(vT[:]))
    nc.sync.dma_start(out=outv_d[:], in_=vout[:])


def kernel(inputs, W):
    global _PROGRAM
    from concourse.bass_utils import run_bass_kernel_spmd

    shared, per_core = _host_prep(np.asarray(inputs), np.asarray(W))
    if _PROGRAM is None:
        _PROGRAM = _build_program()
    in_maps = [{**shared, **pc} for pc in per_core]
    res = run_bass_kernel_spmd(_PROGRAM, in_maps, core_ids=list(range(NCORES)))
    out = np.empty((B, J, OUTW), dtype=np.float32)
    for m, r in enumerate(res.results):
        # out_c[p, ch, j, b] -> c[b, j, ch*128+p]
        c = r["out_c"].transpose(3, 2, 1, 0).reshape(BL, J, I)
        out[m * BL:(m + 1) * BL, :, E:] = c
        out[m * BL:(m + 1) * BL, :, 0:E] = r["out_v"]
    return out


if __name__ == "__main__":
    rng = np.random.default_rng(0)
    x = rng.standard_normal((B, I, D), dtype=np.float32)
    w = rng.standard_normal((J, I, E, D), dtype=np.float32)
    y = kernel(x, w)
    print(y.shape, y.dtype)
